# revision 5
# baseline (speedup 1.0000x reference)
"""Chebyshev graph-conv kernel for Trainium2 (8 NeuronCores, SPMD).

Math: out[b,o,m,t] = sum_{k,c,n} T[k,n,m] * x[b,c,n,t] * Theta[k,c,o]
with T the Chebyshev polynomials of the normalized adjacency (n=24, K=3).

The whole operator collapses into a single 768x768 matrix
    W[(c,n),(o,m)] = sum_k Theta[k,c,o] * T[k,n,m]
applied per batch element to x[b] viewed as (c*n, t) = (768, 512):
    out[b](o*24+m, t) = W.T-contract over rows -> exactly one matmul chain.

W is tiny and computed on host from adj/Theta; x is read once and out
written once. Data-parallel over batch: 64 -> 8 per core. x, W AND the
output ship as fp16 (the PE multiplies 16-bit operands at full rate with
fully-hidden weight loads; fp16 I/O halves HBM traffic both ways; the
fp16 output rounding adds ~3e-4 absmax rel err on top of the ~4e-4 from
fp16 inputs — well inside tolerance). PSUM accumulation is full fp32.
Per core: 8 batch elements, each a 6x6 chain of [128,128]x[128,512]
matmuls accumulated in PSUM. The 288-matmul stream runs at the fp16 PE
roofline (~216.6 ns/MM); everything else is arranged to not add to it:

 - Batch 0 runs i-outer/j-inner across 6 concurrently-open PSUM banks,
   so its first matmul needs only W chunk 0 + x0 chunk 0 (~324 KB)
   instead of the full 2 MB the j-outer order would gate on. Loads are
   issued in exactly that consumption order (W_i, x0_i pairs).
 - Batches 1-7 run j-outer/i-inner (each j's PSUM retires right after
   its 6-MM chain, so the PSUM pool cycles cleanly and the per-batch
   copy/store pipeline overlaps the next batch's matmuls).
 - Two dummy matmuls on a zeroed tile bridge the PE-busy window from
   kernel start to the first real matmul: the PE boots clock-gated at
   1.2 GHz and unthrottles only after ~3.4us of sustained busy, so
   starting that window during the initial load latency shaves ~1us of
   half-clock penalty off the real stream.
"""

import numpy as np

import concourse.mybir as mybir
from concourse import bacc, tile
from concourse.bass_utils import run_bass_kernel_spmd

N_CORES = 8
B, C, NV, T = 64, 32, 24, 512
K = 3
O = 32
CN = C * NV   # 768 contraction rows
OM = O * NV   # 768 output rows
BP = B // N_CORES  # 8 batch elements per core
P = 128
NBLK = CN // P  # 6

_compiled_nc = None
last_result = None  # BassKernelResults from the most recent run (for test.py)


def _build_nc():
    f32 = mybir.dt.float32
    f16 = mybir.dt.float16
    nc = bacc.Bacc("TRN2", target_bir_lowering=False, debug=False,
                   num_devices=N_CORES)
    xs = nc.dram_tensor("xs", [BP, CN, T], f16, kind="ExternalInput")
    w = nc.dram_tensor("w", [CN, OM], f16, kind="ExternalInput")
    out = nc.dram_tensor("out", [BP, OM, T], f16, kind="ExternalOutput")

    wr = w[:].rearrange("(i p) m -> p i m", p=P)

    with tile.TileContext(nc) as tc:
        with (
            tc.tile_pool(name="wpool", bufs=1) as wpool,
            tc.tile_pool(name="xpool", bufs=5) as xpool,
            tc.tile_pool(name="opool", bufs=6) as opool,
            tc.tile_pool(name="psum", bufs=8, space="PSUM") as psum_pool,
        ):
            # HAM warm-up bridge: PE busy from ~kernel start until the
            # first real matmul's operands land (~1.4us later).
            warm = wpool.tile([P, T], f16, tag="warm")
            nc.gpsimd.memset(warm[:], 0.0)
            for wi in range(2):
                wps = psum_pool.tile([P, T], f32, tag="ps", name=f"warm_ps{wi}")
                nc.tensor.matmul(wps[:], warm[:, :P], warm[:], start=True, stop=True)

            # W as 6 chunks of [128 (cn), 768 (om)] in one SBUF tile; x0
            # chunk-wise in lockstep with W so chunk i of both arrives just
            # ahead of batch 0's i-th accumulation round.
            wt = wpool.tile([P, NBLK, OM], f16)
            xt0 = xpool.tile([P, NBLK, T], f16)
            xr0 = xs[0].rearrange("(i p) t -> p i t", p=P)
            for i in range(NBLK):
                nc.sync.dma_start(wt[:, i, :], wr[:, i, :])
                nc.sync.dma_start(xt0[:, i, :], xr0[:, i, :])

            xts = [xt0]
            for b in range(1, BP):
                xt = xpool.tile([P, NBLK, T], f16, tag="xt0")
                xr = xs[b].rearrange("(i p) t -> p i t", p=P)
                nc.sync.dma_start(xt[:], xr)
                xts.append(xt)

            # Batch 0: i-outer across 6 open PSUM banks — first matmul
            # gates only on chunk 0 of W and x0.
            ps0 = [psum_pool.tile([P, T], f32, tag="ps", name=f"ps0_{j}")
                   for j in range(NBLK)]
            for i in range(NBLK):
                for j in range(NBLK):
                    nc.tensor.matmul(
                        ps0[j][:],
                        wt[:, i, j * P:(j + 1) * P],
                        xt0[:, i, :],
                        start=(i == 0),
                        stop=(i == NBLK - 1),
                    )
            ot0 = opool.tile([P, NBLK, T], f16)
            orr0 = out[0].rearrange("(j p) t -> p j t", p=P)
            for j in range(NBLK):
                nc.vector.tensor_copy(ot0[:, j, :], ps0[j][:])
                nc.scalar.dma_start(orr0[:, j, :], ot0[:, j, :])

            # Batches 1-7: j-outer, PSUM bank per j retires immediately.
            for b in range(1, BP):
                xt = xts[b]
                ot = opool.tile([P, NBLK, T], f16)
                orr = out[b].rearrange("(j p) t -> p j t", p=P)
                for j in range(NBLK):
                    ps = psum_pool.tile([P, T], f32)
                    for i in range(NBLK):
                        nc.tensor.matmul(
                            ps[:],
                            wt[:, i, j * P:(j + 1) * P],
                            xt[:, i, :],
                            start=(i == 0),
                            stop=(i == NBLK - 1),
                        )
                    nc.vector.tensor_copy(ot[:, j, :], ps[:])
                    nc.scalar.dma_start(orr[:, j, :], ot[:, j, :])

    nc.compile()
    return nc


def _combined_operator(adj: np.ndarray, Theta: np.ndarray) -> np.ndarray:
    """W[(c,n),(o,m)] = sum_k Theta[k,c,o] * T[k,n,m], fp16, shape (768,768)."""
    adj = np.asarray(adj).astype(np.float32)
    Theta = np.asarray(Theta)
    d = adj.sum(axis=1)
    d_inv_sqrt = np.where(d > 0, 1.0 / np.sqrt(d), 0.0).astype(np.float32)
    L = (adj * d_inv_sqrt[None, :]).T * d_inv_sqrt[None, :]
    Ts = [np.eye(NV, dtype=np.float32), L.astype(np.float32)]
    for _ in range(2, K):
        Ts.append((2.0 * L @ Ts[-1] - Ts[-2]).astype(np.float32))
    Tcheb = np.stack(Ts[:K])  # (K, n, m)
    W = np.einsum("kco,knm->cnom", Theta.astype(np.float32), Tcheb)
    return np.ascontiguousarray(W.reshape(CN, OM), dtype=np.float16)


def kernel(x: np.ndarray, adj: np.ndarray, Theta: np.ndarray) -> np.ndarray:
    global _compiled_nc, last_result
    if _compiled_nc is None:
        _compiled_nc = _build_nc()
    nc = _compiled_nc

    W = _combined_operator(adj, Theta)
    # x: (64, 32, 24, 512) -> per-core shard [8, 768, 512], fp16 (the device
    # matmul consumes fp16 regardless; casting host-side halves HBM reads)
    xf = np.asarray(x).astype(np.float16).reshape(B, CN, T)
    in_maps = [
        {"xs": np.ascontiguousarray(xf[c * BP:(c + 1) * BP]), "w": W}
        for c in range(N_CORES)
    ]
    res = run_bass_kernel_spmd(nc, in_maps, core_ids=list(range(N_CORES)))
    last_result = res
    out = np.concatenate([r["out"] for r in res.results], axis=0)
    return np.ascontiguousarray(out.reshape(B, O, NV, T).astype(np.float32))


# revision 6
# speedup vs baseline: 1.2200x; 1.2200x over previous
"""Chebyshev graph-conv kernel for Trainium2 (8 NeuronCores, SPMD).

Math: out[b,o,m,t] = sum_{k,c,n} T[k,n,m] * x[b,c,n,t] * Theta[k,c,o]
with T the Chebyshev polynomials of the normalized adjacency (n=24, K=3).

The whole operator collapses into a single 768x768 matrix
    W[(c,n),(o,m)] = sum_k Theta[k,c,o] * T[k,n,m]
applied per batch element to x[b] viewed as (c*n, t) = (768, 512):
    out[b](o*24+m, t) = W.T-contract over rows -> exactly one matmul chain.

W is tiny and computed on host from adj/Theta; x is read once and out
written once. Data-parallel over batch: 64 -> 8 per core. x, W AND the
output ship as fp16 (the PE multiplies 16-bit operands at full rate with
fully-hidden weight loads; fp16 I/O halves HBM traffic both ways; the
fp16 output rounding adds ~3e-4 absmax rel err on top of the ~4e-4 from
fp16 inputs — well inside tolerance). PSUM accumulation is full fp32.
Per core: 8 batch elements, each a 6x6 chain of [128,128]x[128,512]
matmuls accumulated in PSUM. The 288-matmul stream runs at the fp16 PE
roofline (~216.6 ns/MM); everything else is arranged to not add to it:

 - Batch 0 runs i-outer/j-inner across 6 concurrently-open PSUM banks,
   so its first matmul needs only W chunk 0 + x0 chunk 0 (~324 KB)
   instead of the full 2 MB the j-outer order would gate on. Loads are
   issued in exactly that consumption order (W_i, x0_i pairs).
 - Batches 1-7 run j-outer/i-inner (each j's PSUM retires right after
   its 6-MM chain, so the PSUM pool cycles cleanly and the per-batch
   copy/store pipeline overlaps the next batch's matmuls).
 - Two dummy matmuls on a zeroed tile bridge the PE-busy window from
   kernel start to the first real matmul: the PE boots clock-gated at
   1.2 GHz and unthrottles only after ~3.4us of sustained busy, so
   starting that window during the initial load latency shaves ~1us of
   half-clock penalty off the real stream.
"""

import numpy as np

import concourse.mybir as mybir
from concourse import bacc, tile
from concourse.bass_utils import run_bass_kernel_spmd

N_CORES = 8
B, C, NV, T = 64, 32, 24, 512
K = 3
O = 32
CN = C * NV   # 768 contraction rows
OM = O * NV   # 768 output rows
BP = B // N_CORES  # 8 batch elements per core
P = 128
NBLK = CN // P  # 6

_compiled_nc = None
last_result = None  # BassKernelResults from the most recent run (for test.py)


def _build_nc():
    f32 = mybir.dt.float32
    f16 = mybir.dt.float16
    nc = bacc.Bacc("TRN2", target_bir_lowering=False, debug=False,
                   num_devices=N_CORES)
    xs = nc.dram_tensor("xs", [BP, CN, T], f16, kind="ExternalInput")
    w = nc.dram_tensor("w", [CN, OM], f16, kind="ExternalInput")
    out = nc.dram_tensor("out", [BP, OM, T], f16, kind="ExternalOutput")

    wr = w[:].rearrange("(i p) m -> p i m", p=P)

    with tile.TileContext(nc) as tc:
        with (
            tc.tile_pool(name="wpool", bufs=1) as wpool,
            tc.tile_pool(name="xpool", bufs=5) as xpool,
            tc.tile_pool(name="opool", bufs=6) as opool,
            tc.tile_pool(name="psum", bufs=8, space="PSUM") as psum_pool,
        ):
            # HAM warm-up bridge: PE busy from ~kernel start until the
            # first real matmul's operands land (~1.4us later).
            warm = wpool.tile([P, T], f16, tag="warm")
            nc.gpsimd.memset(warm[:], 0.0)
            for wi in range(2):
                wps = psum_pool.tile([P, T], f32, tag="ps", name=f"warm_ps{wi}")
                nc.tensor.matmul(wps[:], warm[:, :P], warm[:], start=True, stop=True)

            # W as 6 chunks of [128 (cn), 768 (om)] in one SBUF tile, loaded
            # on the Scalar HWDGE ring (idle until the first stores ~10us
            # later) so the W issues don't serialize with the x issues on
            # the Sync ring; x0 chunk-wise on Sync in consumption order.
            # Chunk i of W and x0 both arrive just ahead of batch 0's i-th
            # accumulation round.
            wt = wpool.tile([P, NBLK, OM], f16)
            xt0 = xpool.tile([P, NBLK, T], f16)
            xr0 = xs[0].rearrange("(i p) t -> p i t", p=P)
            for i in range(NBLK):
                nc.sync.dma_start(xt0[:, i, :], xr0[:, i, :])
                nc.scalar.dma_start(wt[:, i, :], wr[:, i, :])

            # Batches 1-7 load as two half-tiles each so their first
            # accumulation rounds gate on the first half only.
            xts = [xt0]
            for b in range(1, BP):
                xt = xpool.tile([P, NBLK, T], f16, tag="xt0")
                xr = xs[b].rearrange("(i p) t -> p i t", p=P)
                h = NBLK // 2
                nc.sync.dma_start(xt[:, :h, :], xr[:, :h, :])
                nc.sync.dma_start(xt[:, h:, :], xr[:, h:, :])
                xts.append(xt)

            # Batch 0: i-outer across 6 open PSUM banks — first matmul
            # gates only on chunk 0 of W and x0.
            ps0 = [psum_pool.tile([P, T], f32, tag="ps", name=f"ps0_{j}")
                   for j in range(NBLK)]
            for i in range(NBLK):
                for j in range(NBLK):
                    nc.tensor.matmul(
                        ps0[j][:],
                        wt[:, i, j * P:(j + 1) * P],
                        xt0[:, i, :],
                        start=(i == 0),
                        stop=(i == NBLK - 1),
                    )
            ot0 = opool.tile([P, NBLK, T], f16)
            orr0 = out[0].rearrange("(j p) t -> p j t", p=P)
            for j in range(NBLK):
                nc.vector.tensor_copy(ot0[:, j, :], ps0[j][:])
                nc.scalar.dma_start(orr0[:, j, :], ot0[:, j, :])

            # Batches 1-7: j-outer, PSUM bank per j retires immediately.
            for b in range(1, BP):
                xt = xts[b]
                ot = opool.tile([P, NBLK, T], f16)
                orr = out[b].rearrange("(j p) t -> p j t", p=P)
                for j in range(NBLK):
                    ps = psum_pool.tile([P, T], f32)
                    for i in range(NBLK):
                        nc.tensor.matmul(
                            ps[:],
                            wt[:, i, j * P:(j + 1) * P],
                            xt[:, i, :],
                            start=(i == 0),
                            stop=(i == NBLK - 1),
                        )
                    nc.vector.tensor_copy(ot[:, j, :], ps[:])
                    nc.scalar.dma_start(orr[:, j, :], ot[:, j, :])

    nc.compile()
    return nc


def _combined_operator(adj: np.ndarray, Theta: np.ndarray) -> np.ndarray:
    """W[(c,n),(o,m)] = sum_k Theta[k,c,o] * T[k,n,m], fp16, shape (768,768)."""
    adj = np.asarray(adj).astype(np.float32)
    Theta = np.asarray(Theta)
    d = adj.sum(axis=1)
    d_inv_sqrt = np.where(d > 0, 1.0 / np.sqrt(d), 0.0).astype(np.float32)
    L = (adj * d_inv_sqrt[None, :]).T * d_inv_sqrt[None, :]
    Ts = [np.eye(NV, dtype=np.float32), L.astype(np.float32)]
    for _ in range(2, K):
        Ts.append((2.0 * L @ Ts[-1] - Ts[-2]).astype(np.float32))
    Tcheb = np.stack(Ts[:K])  # (K, n, m)
    W = np.einsum("kco,knm->cnom", Theta.astype(np.float32), Tcheb)
    return np.ascontiguousarray(W.reshape(CN, OM), dtype=np.float16)


def kernel(x: np.ndarray, adj: np.ndarray, Theta: np.ndarray) -> np.ndarray:
    global _compiled_nc, last_result
    if _compiled_nc is None:
        _compiled_nc = _build_nc()
    nc = _compiled_nc

    W = _combined_operator(adj, Theta)
    # x: (64, 32, 24, 512) -> per-core shard [8, 768, 512], fp16 (the device
    # matmul consumes fp16 regardless; casting host-side halves HBM reads)
    xf = np.asarray(x).astype(np.float16).reshape(B, CN, T)
    in_maps = [
        {"xs": np.ascontiguousarray(xf[c * BP:(c + 1) * BP]), "w": W}
        for c in range(N_CORES)
    ]
    res = run_bass_kernel_spmd(nc, in_maps, core_ids=list(range(N_CORES)))
    last_result = res
    out = np.concatenate([r["out"] for r in res.results], axis=0)
    return np.ascontiguousarray(out.reshape(B, O, NV, T).astype(np.float32))


# revision 9
# speedup vs baseline: 1.2241x; 1.0034x over previous
"""Chebyshev graph-conv kernel for Trainium2 (8 NeuronCores, SPMD).

Math: out[b,o,m,t] = sum_{k,c,n} T[k,n,m] * x[b,c,n,t] * Theta[k,c,o]
with T the Chebyshev polynomials of the normalized adjacency (n=24, K=3).

The whole operator collapses into a single 768x768 matrix
    W[(c,n),(o,m)] = sum_k Theta[k,c,o] * T[k,n,m]
applied per batch element to x[b] viewed as (c*n, t) = (768, 512):
    out[b](o*24+m, t) = W.T-contract over rows -> exactly one matmul chain.

W is tiny and computed on host from adj/Theta; x is read once and out
written once. Data-parallel over batch: 64 -> 8 per core. x, W AND the
output ship as fp16 (the PE multiplies 16-bit operands at full rate with
fully-hidden weight loads; fp16 I/O halves HBM traffic both ways; the
fp16 output rounding adds ~3e-4 absmax rel err on top of the ~4e-4 from
fp16 inputs — well inside tolerance). PSUM accumulation is full fp32.
Per core: 8 batch elements, each a 6x6 chain of [128,128]x[128,512]
matmuls accumulated in PSUM. The 288-matmul stream runs at the fp16 PE
roofline (~216.6 ns/MM); everything else is arranged to not add to it:

 - Batch 0 runs i-outer/j-inner across 6 concurrently-open PSUM banks,
   so its first matmul needs only W chunk 0 + x0 chunk 0 (~324 KB)
   instead of the full 2 MB the j-outer order would gate on. Loads are
   issued in exactly that consumption order (W_i, x0_i pairs).
 - Batches 1-7 run j-outer/i-inner (each j's PSUM retires right after
   its 6-MM chain, so the PSUM pool cycles cleanly and the per-batch
   copy/store pipeline overlaps the next batch's matmuls).
 - Two dummy matmuls on a zeroed tile bridge the PE-busy window from
   kernel start to the first real matmul: the PE boots clock-gated at
   1.2 GHz and unthrottles only after ~3.4us of sustained busy, so
   starting that window during the initial load latency shaves ~1us of
   half-clock penalty off the real stream.
"""

import numpy as np

import concourse.mybir as mybir
from concourse import bacc, tile
from concourse.bass_utils import run_bass_kernel_spmd

N_CORES = 8
B, C, NV, T = 64, 32, 24, 512
K = 3
O = 32
CN = C * NV   # 768 contraction rows
OM = O * NV   # 768 output rows
BP = B // N_CORES  # 8 batch elements per core
P = 128
NBLK = CN // P  # 6

_compiled_nc = None
last_result = None  # BassKernelResults from the most recent run (for test.py)


def _build_nc():
    f32 = mybir.dt.float32
    f16 = mybir.dt.float16
    nc = bacc.Bacc("TRN2", target_bir_lowering=False, debug=False,
                   num_devices=N_CORES)
    xs = nc.dram_tensor("xs", [BP, CN, T], f16, kind="ExternalInput")
    w = nc.dram_tensor("w", [CN, OM], f16, kind="ExternalInput")
    out = nc.dram_tensor("out", [BP, OM, T], f16, kind="ExternalOutput")

    wr = w[:].rearrange("(i p) m -> p i m", p=P)

    with tile.TileContext(nc) as tc:
        with (
            tc.tile_pool(name="wpool", bufs=1) as wpool,
            tc.tile_pool(name="xpool", bufs=5) as xpool,
            tc.tile_pool(name="opool", bufs=6) as opool,
            tc.tile_pool(name="psum", bufs=8, space="PSUM") as psum_pool,
        ):
            # HAM warm-up bridge: PE busy from ~kernel start until the
            # first real matmul's operands land (~1.4us later).
            warm = wpool.tile([P, T], f16, tag="warm")
            nc.gpsimd.memset(warm[:], 0.0)
            for wi in range(2):
                wps = psum_pool.tile([P, T], f32, tag="ps", name=f"warm_ps{wi}")
                for _ in range(2):
                    nc.tensor.matmul(wps[:], warm[:, :P], warm[:],
                                     start=True, stop=True)

            # W as 6 chunks of [128 (cn), 768 (om)] in one SBUF tile, loaded
            # on the Scalar HWDGE ring (idle until the first stores ~10us
            # later) so the W issues don't serialize with the x issues on
            # the Sync ring; x0 chunk-wise on Sync in consumption order.
            # Chunk i of W and x0 both arrive just ahead of batch 0's i-th
            # accumulation round.
            wt = wpool.tile([P, NBLK, OM], f16)
            xt0 = xpool.tile([P, NBLK, T], f16)
            xr0 = xs[0].rearrange("(i p) t -> p i t", p=P)
            # W chunk 0 splits off its first [128,128] j-slice (32 KB) so the
            # very first matmul gates on ~160 KB instead of ~324 KB.
            nc.sync.dma_start(xt0[:, 0, :], xr0[:, 0, :])
            nc.scalar.dma_start(wt[:, 0, 0:P], wr[:, 0, 0:P])
            nc.scalar.dma_start(wt[:, 0, P:], wr[:, 0, P:])
            for i in range(1, NBLK):
                nc.sync.dma_start(xt0[:, i, :], xr0[:, i, :])
                nc.scalar.dma_start(wt[:, i, :], wr[:, i, :])

            # Batches 1-7 load as two half-tiles each so their first
            # accumulation rounds gate on the first half only.
            xts = [xt0]
            for b in range(1, BP):
                xt = xpool.tile([P, NBLK, T], f16, tag="xt0")
                xr = xs[b].rearrange("(i p) t -> p i t", p=P)
                h = NBLK // 2
                nc.sync.dma_start(xt[:, :h, :], xr[:, :h, :])
                nc.sync.dma_start(xt[:, h:, :], xr[:, h:, :])
                xts.append(xt)

            # Batch 0: i-outer across 6 open PSUM banks — first matmul
            # gates only on chunk 0 of W and x0.
            ps0 = [psum_pool.tile([P, T], f32, tag="ps", name=f"ps0_{j}")
                   for j in range(NBLK)]
            for i in range(NBLK):
                for j in range(NBLK):
                    nc.tensor.matmul(
                        ps0[j][:],
                        wt[:, i, j * P:(j + 1) * P],
                        xt0[:, i, :],
                        start=(i == 0),
                        stop=(i == NBLK - 1),
                    )
            ot0 = opool.tile([P, NBLK, T], f16)
            orr0 = out[0].rearrange("(j p) t -> p j t", p=P)
            for j in range(NBLK):
                nc.vector.tensor_copy(ot0[:, j, :], ps0[j][:])
                nc.scalar.dma_start(orr0[:, j, :], ot0[:, j, :])

            # Batches 1-7: j-outer, PSUM bank per j retires immediately.
            for b in range(1, BP):
                xt = xts[b]
                ot = opool.tile([P, NBLK, T], f16)
                orr = out[b].rearrange("(j p) t -> p j t", p=P)
                for j in range(NBLK):
                    ps = psum_pool.tile([P, T], f32)
                    for i in range(NBLK):
                        nc.tensor.matmul(
                            ps[:],
                            wt[:, i, j * P:(j + 1) * P],
                            xt[:, i, :],
                            start=(i == 0),
                            stop=(i == NBLK - 1),
                        )
                    if b == BP - 1 and j == NBLK - 1:
                        # Final block: halve the copy+store and put the
                        # second store on the (idle) Sync ring so the
                        # end-of-kernel drain pipelines instead of
                        # serializing copy -> issue -> full transfer.
                        h = T // 2
                        nc.vector.tensor_copy(ot[:, j, :h], ps[:, :h])
                        nc.scalar.dma_start(orr[:, j, :h], ot[:, j, :h])
                        nc.vector.tensor_copy(ot[:, j, h:], ps[:, h:])
                        nc.sync.dma_start(orr[:, j, h:], ot[:, j, h:])
                    else:
                        nc.vector.tensor_copy(ot[:, j, :], ps[:])
                        nc.scalar.dma_start(orr[:, j, :], ot[:, j, :])

    nc.compile()
    return nc


def _combined_operator(adj: np.ndarray, Theta: np.ndarray) -> np.ndarray:
    """W[(c,n),(o,m)] = sum_k Theta[k,c,o] * T[k,n,m], fp16, shape (768,768)."""
    adj = np.asarray(adj).astype(np.float32)
    Theta = np.asarray(Theta)
    d = adj.sum(axis=1)
    d_inv_sqrt = np.where(d > 0, 1.0 / np.sqrt(d), 0.0).astype(np.float32)
    L = (adj * d_inv_sqrt[None, :]).T * d_inv_sqrt[None, :]
    Ts = [np.eye(NV, dtype=np.float32), L.astype(np.float32)]
    for _ in range(2, K):
        Ts.append((2.0 * L @ Ts[-1] - Ts[-2]).astype(np.float32))
    Tcheb = np.stack(Ts[:K])  # (K, n, m)
    W = np.einsum("kco,knm->cnom", Theta.astype(np.float32), Tcheb)
    return np.ascontiguousarray(W.reshape(CN, OM), dtype=np.float16)


def kernel(x: np.ndarray, adj: np.ndarray, Theta: np.ndarray) -> np.ndarray:
    global _compiled_nc, last_result
    if _compiled_nc is None:
        _compiled_nc = _build_nc()
    nc = _compiled_nc

    W = _combined_operator(adj, Theta)
    # x: (64, 32, 24, 512) -> per-core shard [8, 768, 512], fp16 (the device
    # matmul consumes fp16 regardless; casting host-side halves HBM reads)
    xf = np.asarray(x).astype(np.float16).reshape(B, CN, T)
    in_maps = [
        {"xs": np.ascontiguousarray(xf[c * BP:(c + 1) * BP]), "w": W}
        for c in range(N_CORES)
    ]
    res = run_bass_kernel_spmd(nc, in_maps, core_ids=list(range(N_CORES)))
    last_result = res
    out = np.concatenate([r["out"] for r in res.results], axis=0)
    return np.ascontiguousarray(out.reshape(B, O, NV, T).astype(np.float32))
